# revision 45
# baseline (speedup 1.0000x reference)
"""Causal self-attention (B=2, T=2048, C=1024, H=16) on 8 TRN2 NeuronCores.

Sharding: tensor-parallel over heads. Each core owns 2 heads: it computes
q/k/v projections for its 128 feature columns, full causal attention for its
(batch, head) pairs, and a partial output projection against its 128 rows of
w_proj. The 8 partial [4096, 1024] outputs are summed on host and b_proj is
added once during that reduction.

v4: engine-specialized, bank-interleaved pipeline (from v3's trace):

  * ScalarE runs exp activations plus half the y evacuations only; causal
    masks + the attn assembly DMA go to the otherwise-idle GPSIMD engine;
  * every accumulation chain alternates between two PSUM banks (q/k
    projections pairwise-interleaved, V's four token-tile chains split
    across two tiles) so back-to-back matmuls pipeline their ~165ns
    fill/drain instead of serializing it;
  * the V bias is applied by a tensor_tensor add against a pre-broadcast
    bias tile during PSUM evacuation (no K=1 bias matmuls);
  * the final entry's normalize/projection runs inline with split-head
    (K=64 x2) projection matmuls -- no SBUF->SBUF assembly DMA on the tail
    critical path -- and warm dummy matmuls keep the PE p-state at full
    clock through the normalize latency;
  * first x tile DMA'd in four pieces so the first qkv chain starts ~1us in.

Softmax skips the max-subtraction: scores are ~N(0,1) (bounded ~+-6), far
inside fp32 exp range. Denominators fall out of the AV matmul via a ones
column appended to V per head; normalization happens on the tiny attn^T
tensor via a K=1 broadcast matmul + fast reciprocal.
"""

from collections import deque

import numpy as np
import ml_dtypes

import concourse.bass as bass
import concourse.mybir as mybir
import concourse.tile as tile
from concourse import bacc
from concourse.bass_utils import run_bass_kernel_spmd

F32 = mybir.dt.float32
BF16 = mybir.dt.bfloat16
EXP = mybir.ActivationFunctionType.Exp

B, T, C = 2, 2048, 1024
H, DH = 16, 64
NCORES = 8
FPC = (H // NCORES) * DH  # 128 q/k/v feature columns per core (2 heads)
N = B * T                 # 4096 tokens
NTT = N // 128            # 32 global 128-token tiles
NCT = C // 128            # 8 contraction tiles
SCALE = DH ** -0.5

# feature flags (HW bisect)
INTERLEAVE_QKV = True  # q/k and V chains alternate PSUM banks
BIAS_IN_EVAC = True    # V bias folded into evacuation add (no K=1 matmuls)
GPSIMD_OFFLOAD = True  # masks + norm-muls + assembly DMA on GPSIMD
TAIL_INLINE = True     # last entry: inline split-head proj + warm fill
LAG2 = True            # AV issued two steps after its S (else one step)

_CACHE = {}


def _build():
    nc = bacc.Bacc(
        "TRN2",
        target_bir_lowering=False,
        debug=False,
        enable_asserts=True,
        num_devices=NCORES,
    )
    xT = nc.dram_tensor("xT", [C, N], BF16, kind="ExternalInput").ap()
    wq = nc.dram_tensor("wq", [C, FPC], BF16, kind="ExternalInput").ap()
    wk = nc.dram_tensor("wk", [C, FPC], BF16, kind="ExternalInput").ap()
    wv = nc.dram_tensor("wv", [C, FPC], BF16, kind="ExternalInput").ap()
    bq = nc.dram_tensor("bq", [FPC, 1], F32, kind="ExternalInput").ap()
    bk = nc.dram_tensor("bk", [FPC, 1], F32, kind="ExternalInput").ap()
    bv = nc.dram_tensor("bv", [1, FPC], BF16, kind="ExternalInput").ap()
    wp = nc.dram_tensor("wp", [FPC, C], BF16, kind="ExternalInput").ap()
    y = nc.dram_tensor("y", [N, C], BF16, kind="ExternalOutput").ap()

    with tile.TileContext(nc) as tc:
        with (
            tc.tile_pool(name="const", bufs=1) as cst,
            tc.tile_pool(name="qkvt", bufs=1) as qkvt,
            tc.tile_pool(name="xin", bufs=3) as xin,
            tc.tile_pool(name="ptile", bufs=6) as ptile,
            tc.tile_pool(name="attn", bufs=12) as attnp,
            tc.tile_pool(name="yout", bufs=6) as yout,
            tc.tile_pool(name="small", bufs=3) as small,
            tc.tile_pool(name="ps_s", bufs=2, space="PSUM") as ps_s,
            tc.tile_pool(name="ps_av", bufs=2, space="PSUM") as ps_av,
            tc.tile_pool(name="ps_misc", bufs=2, space="PSUM") as ps_misc,
        ):
            # ---- input DMAs; first x tile split in quarters so the very
            # first qkv matmuls can start during the p-state ramp ----
            xT_view = xT.rearrange("(ct p) t -> p ct t", p=128)
            xt_tiles = {}

            def dma_xt(b, tj, split=False):
                xt = xin.tile([128, NCT, 512], BF16, tag="xt", name="xt")
                t0 = b * T + tj * 512
                if split:
                    for i in range(4):
                        nc.sync.dma_start(
                            out=xt[:, 2 * i : 2 * i + 2, :],
                            in_=xT_view[:, 2 * i : 2 * i + 2, t0 : t0 + 512],
                        )
                else:
                    nc.sync.dma_start(out=xt, in_=xT_view[:, :, t0 : t0 + 512])
                xt_tiles[(b, tj)] = xt

            # ---- weights / biases. DMA *issue* costs 0.6-3.5us of the
            # issuing engine's time, so spread issues across idle engine
            # queues: sync carries wq + the x stream (critical path), the
            # other engines carry everything else in parallel ----
            w_sb = {}
            w_sb["q"] = cst.tile([128, NCT, FPC], BF16, tag="wq", name="wq")
            nc.sync.dma_start(
                out=w_sb["q"], in_=wq.rearrange("(ct p) f -> p ct f", p=128)
            )
            dma_xt(0, 0, split=True)
            for name, wap, eng in (("k", wk, nc.scalar), ("v", wv, nc.scalar)):
                w_sb[name] = cst.tile(
                    [128, NCT, FPC], BF16, tag=f"w{name}", name=f"w{name}"
                )
                eng.dma_start(
                    out=w_sb[name], in_=wap.rearrange("(ct p) f -> p ct f", p=128)
                )
            b_sb = {}
            for name, bap in (("q", bq), ("k", bk)):
                b_sb[name] = cst.tile([FPC, 1], F32, tag=f"b{name}", name=f"b{name}")
                nc.scalar.dma_start(out=b_sb[name], in_=bap)
            bv_sb = cst.tile([1, FPC], BF16, tag="bv", name="bv")
            nc.gpsimd.dma_start(out=bv_sb, in_=bv)
            wp_sb = cst.tile([FPC, C], BF16, tag="wp", name="wp")
            nc.gpsimd.dma_start(out=wp_sb, in_=wp)
            # head-1 rows of w_proj staged at partitions 0-63 so the tail's
            # split-head projection has matching base partitions
            wp_h1 = cst.tile([64, C], BF16, tag="wph1", name="wph1")
            nc.gpsimd.dma_start(out=wp_h1, in_=wp[64:128, :])

            # ---- constants; warm-up deps (onesv/warmb) first so the PE's
            # dummy ramp matmuls can start as early as possible ----
            onesf = cst.tile([128, 128], F32, tag="onesf", name="onesf")
            nc.vector.memset(onesf, 1.0)
            onesv = cst.tile([1, 128], BF16, tag="onesv", name="onesv")
            nc.vector.tensor_copy(out=onesv, in_=onesf[0:1, :])
            warm = cst.tile([1, 512], F32, tag="warm", name="warm")
            nc.vector.memset(warm, 1.0)
            warmb = cst.tile([1, 512], BF16, tag="warmb", name="warmb")
            nc.vector.tensor_copy(out=warmb, in_=warm)

            def warm_fill(n, pool=None, width=512):
                for _ in range(n):
                    p = pool or ps_misc
                    wps = p.tile([128, width], F32, tag="misc" if p is ps_misc else "s",
                                 name="warmup")
                    nc.tensor.matmul(
                        wps, onesv, warmb[:, 0:width], start=True, stop=True
                    )

            warm_fill(10)
            # lower-triangle mask (keep iff q-col >= k-partition), duplicated
            # side by side so one strided multiply covers both heads
            mtf = cst.tile([128, 128], F32, tag="mtf", name="mtf")
            nc.vector.memset(mtf, 1.0)
            nc.gpsimd.affine_select(
                out=mtf,
                in_=mtf,
                compare_op=mybir.AluOpType.is_ge,
                fill=0.0,
                base=0,
                pattern=[[1, 128]],
                channel_multiplier=-1,
            )
            M_tri2 = cst.tile([128, 2, 128], BF16, tag="mtri", name="mtri")
            nc.vector.tensor_copy(out=M_tri2[:, 0, :], in_=mtf)
            nc.vector.tensor_copy(out=M_tri2[:, 1, :], in_=mtf)
            # ones row at partition 64 (stationary of the K=1 denominator
            # broadcast matmul; partition 64 = where AV's ones-column lands)
            ones64 = cst.tile([128, 64], BF16, tag="ones64", name="ones64")
            nc.vector.tensor_copy(out=ones64[64:65, :], in_=onesf[64:65, 0:64])
            bv_bc = cst.tile([128, 2, 2, 64], BF16, tag="bvbc", name="bvbc")

            def build_bv_bc():
                # V bias broadcast to all 128 token partitions, duplicated
                # for two token-tiles: [128, tt2, head2, 64] so one
                # tensor_tensor add covers a whole pv evacuation
                bvp = ps_misc.tile([128, 128], F32, tag="misc", name="bvp")
                nc.tensor.matmul(bvp, onesv, bv_sb, start=True, stop=True)
                nc.vector.tensor_copy(
                    out=bv_bc[:, 0, :, :],
                    in_=bvp.rearrange("p (two c) -> p two c", two=2),
                )
                nc.vector.tensor_copy(out=bv_bc[:, 1, :, :], in_=bv_bc[:, 0, :, :])

            # ---- persistent activations ----
            QT = qkvt.tile([FPC, N], BF16, tag="QT", name="QT")
            KT = qkvt.tile([FPC, N], BF16, tag="KT", name="KT")
            # V with a ones column per head: per 128-token tile block of 130
            # cols: [64 V_h0 | 1 | 64 V_h1 | 1]
            V = qkvt.tile([128, NTT * 130], BF16, tag="V", name="V")
            V_blk = V.rearrange("p (kt two c) -> p kt two c", two=2, c=65)
            nc.vector.tensor_copy(out=V_blk[:, :, 0, 64], in_=onesf[:, 0:NTT])
            nc.vector.tensor_copy(out=V_blk[:, :, 1, 64], in_=onesf[:, 0:NTT])

            # ---- deferred-work queue: (deadline_entry, thunk) ----
            pending = deque()

            def drain(ci):
                rest = [item for item in pending if item[0] > ci]
                due = [item for item in pending if item[0] <= ci]
                pending.clear()
                pending.extend(rest)
                for _, th in due:
                    th()

            def pop_some(steps_left):
                n = 2 if len(pending) > steps_left else (1 if pending else 0)
                for _ in range(min(n, len(pending))):
                    pending.popleft()[1]()

            # ---- qkv projection bundle -> thunks ----
            # q and k accumulate pairwise-interleaved into two PSUM tiles so
            # consecutive matmuls hit different banks and pipeline; same for
            # V's four token-tile chains (tt0/tt1 in one tile, tt2/tt3 in the
            # other, ct-major emission alternating tiles).
            def qkv_thunks(b, tj):
                t0 = b * T + tj * 512
                qk_box = {}
                v_box = {}

                def qk_half(first):
                    def th():
                        xt = xt_tiles[(b, tj)]
                        if first:
                            qk_box["q"] = ps_misc.tile(
                                [128, 512], F32, tag="misc", name="accq"
                            )
                            qk_box["k"] = ps_misc.tile(
                                [128, 512], F32, tag="misc", name="acck"
                            )
                        cts = range(0, 4) if first else range(4, NCT)
                        for ct in cts:
                            for name in ("q", "k"):
                                nc.tensor.matmul(
                                    qk_box[name],
                                    w_sb[name][:, ct, :],
                                    xt[:, ct, :],
                                    start=(ct == 0),
                                    stop=(ct == NCT - 1),
                                    skip_group_check=True,
                                )
                        if not first:
                            nc.vector.tensor_scalar_add(
                                QT[:, t0 : t0 + 512], qk_box["q"], b_sb["q"]
                            )
                            nc.vector.tensor_scalar_add(
                                KT[:, t0 : t0 + 512], qk_box["k"], b_sb["k"]
                            )

                    return th

                def v_half(first):
                    # a PSUM bank supports only ONE open accumulation group:
                    # interleave strictly ACROSS the two tiles (banks), with
                    # each pass running complete chains (tt0/tt2 first, then
                    # tt1/tt3 in the tiles' second halves)
                    def th():
                        xt = xt_tiles[(b, tj)]
                        if first:
                            v_box["a"] = ps_misc.tile(
                                [128, 256], F32, tag="misc", name="pva"
                            )
                            v_box["b"] = ps_misc.tile(
                                [128, 256], F32, tag="misc", name="pvb"
                            )
                        pairs = (
                            ((0, v_box["a"], slice(0, 128)),
                             (2, v_box["b"], slice(0, 128)))
                            if first
                            else ((1, v_box["a"], slice(128, 256)),
                                  (3, v_box["b"], slice(128, 256)))
                        )
                        for ct in range(NCT):
                            for tt, tile_, sl in pairs:
                                nc.tensor.matmul(
                                    tile_[:, sl],
                                    xt[:, ct, tt * 128 : (tt + 1) * 128],
                                    w_sb["v"][:, ct, :],
                                    start=(ct == 0),
                                    stop=(ct == NCT - 1),
                                    skip_group_check=True,
                                )
                        if not first:
                            gt = b * 16 + tj * 4
                            for key, g0 in (("a", gt), ("b", gt + 2)):
                                pv = v_box[key].rearrange(
                                    "p (tt two c) -> p tt two c", tt=2, two=2, c=64
                                )
                                nc.vector.tensor_tensor(
                                    out=V_blk[:, g0 : g0 + 2, :, 0:64],
                                    in0=pv,
                                    in1=bv_bc,
                                    op=mybir.AluOpType.add,
                                )

                    return th

                return [qk_half(True), qk_half(False), v_half(True), v_half(False)]

            # ---- softmax-normalize + output projection (deferred) ----
            def bc_norm_thunk(avs, q_w, box):
                def th():
                    attn_t = attnp.tile([128, 512], BF16, tag="attn", name="attn")[:, 0:q_w]
                    attn1 = attnp.tile([64, 512], BF16, tag="attn1", name="attn1")[:, 0:q_w]
                    for h in range(2):
                        bcp = ps_misc.tile([64, 512], F32, tag="misc", name="bc")[:, 0:q_w]
                        nc.tensor.matmul(
                            bcp, ones64[64:65, :], avs[h][64:65, :],
                            start=True, stop=True,
                        )
                        rbc = small.tile([64, 512], F32, tag="rbc", name="rbc")[:, 0:q_w]
                        nc.vector.reciprocal_approx_fast(rbc, bcp)
                        tgt = attn_t[0:64, :] if h == 0 else attn1
                        nc.vector.tensor_mul(tgt, avs[h][0:64, :], rbc)
                    # head-1 half to partitions 64..127 (SBUF->SBUF DMA is the
                    # only cheap cross-partition path); issued from GPSIMD so
                    # ScalarE stays a pure exp stream
                    dma_eng = nc.gpsimd if GPSIMD_OFFLOAD else nc.scalar
                    dma_eng.dma_start(out=attn_t[64:128, :], in_=attn1)
                    box["attn"] = attn_t

                return th

            def proj_thunk(b, qc, q_lo, tt, cc, box, ybox):
                def th():
                    attn_t = box["attn"]
                    yp = ps_misc.tile([128, 512], F32, tag="misc", name="yp")
                    nc.tensor.matmul(
                        yp,
                        attn_t[:, tt * 128 : (tt + 1) * 128],
                        wp_sb[:, cc * 512 : (cc + 1) * 512],
                        start=True,
                        stop=True,
                    )
                    if cc == 0:
                        ybox["ysb"] = yout.tile([128, C], BF16, tag="ysb", name="ysb")
                    ysb = ybox["ysb"]
                    # evacuation alternates DVE / ScalarE so neither becomes
                    # the bottleneck
                    if cc == 0:
                        nc.vector.tensor_copy(
                            out=ysb[:, cc * 512 : (cc + 1) * 512], in_=yp
                        )
                    else:
                        nc.scalar.copy(ysb[:, cc * 512 : (cc + 1) * 512], yp)
                        t0 = b * T + qc * 512 + q_lo + tt * 128
                        nc.sync.dma_start(out=y[t0 : t0 + 128, :], in_=ysb)

                return th

            # ---- attention chunk (columns [q_lo, q_lo+q_w) of q-chunk qc):
            # S -> exp/mask -> AV (lag-2), deferred thunks sprinkled ----
            def emit_chunk(ci, b, qc, q_lo, q_w, tail=False):
                drain(ci)
                q0 = b * T + qc * 512 + q_lo
                hi = qc * 512 + q_lo + q_w  # global end col within the batch
                nkt = hi // 128
                av = [
                    ps_av.tile([65, 512], F32, tag="av", name="av")[:, 0:q_w]
                    for _ in range(2)
                ]

                def emit_av(pt, lo, kt):
                    gkt = b * 16 + kt
                    for h in range(2):
                        nc.tensor.matmul(
                            av[h][:, lo:q_w] if lo else av[h],
                            V[:, 130 * gkt + 65 * h : 130 * gkt + 65 * h + 65],
                            pt[:, h * q_w + lo : (h + 1) * q_w],
                            start=(kt == 0),
                            stop=(kt == nkt - 1),
                            skip_group_check=True,
                        )

                backlog = deque()
                for kt in range(nkt):
                    # valid cols of this k-tile within [q_lo, q_lo+q_w)
                    lo = min(max(kt * 128 - (qc * 512 + q_lo), 0), q_w)
                    diag = kt * 128 >= qc * 512 + q_lo  # triangle block here
                    k0 = b * T + kt * 128
                    # both heads in one S tile: the two halves land in
                    # different PSUM banks (512 f32 cols each) so the two
                    # matmuls pipeline, and one exp instruction (262ns fixed
                    # cost) covers both heads
                    s = ps_s.tile([128, 1024], F32, tag="s", name="s")[:, 0 : 2 * q_w]
                    for h in range(2):
                        nc.tensor.matmul(
                            s[:, h * q_w + lo : (h + 1) * q_w],
                            KT[64 * h : 64 * h + 64, k0 : k0 + 128],
                            QT[64 * h : 64 * h + 64, q0 + lo : q0 + q_w],
                            start=True,
                            stop=True,
                        )
                    pt = ptile.tile([128, 1024], BF16, tag="pt", name="pt")[:, 0 : 2 * q_w]
                    if lo == 0:
                        nc.scalar.activation(out=pt, in_=s, func=EXP, scale=SCALE)
                    else:
                        sv = s.rearrange("p (two c) -> p two c", two=2)
                        pv_ = pt.rearrange("p (two c) -> p two c", two=2)
                        nc.scalar.activation(
                            out=pv_[:, :, lo:q_w],
                            in_=sv[:, :, lo:q_w],
                            func=EXP,
                            scale=SCALE,
                        )
                    if diag:  # triangle crossing block, both heads (DVE:
                        # GPSIMD computes this wrong and is 2x slower)
                        ptv = pt.rearrange("p (two c) -> p two c", two=2)
                        nc.vector.tensor_mul(
                            ptv[:, :, lo : lo + 128],
                            ptv[:, :, lo : lo + 128],
                            M_tri2,
                        )
                    pop_some(nkt - 1 - kt)
                    backlog.append((pt, lo, kt))
                    if len(backlog) > (3 if LAG2 else 1):
                        emit_av(*backlog.popleft())
                while backlog:
                    emit_av(*backlog.popleft())
                # evacuate AV PSUM (numerators + denominators) to SBUF bf16;
                # the tail uses ScalarE (idle by then) so the DVE recip/mul
                # chain starts sooner
                avs = []
                for h in range(2):
                    a = attnp.tile([65, 512], BF16, tag="avs", name="avs")[:, 0:q_w]
                    if tail:
                        nc.scalar.copy(a, av[h])
                    else:
                        nc.vector.tensor_copy(a, av[h])
                    avs.append(a)
                return avs

            # ---- tail: the last entry's normalize + projection, inline,
            # with split-head (K=64 x2) proj matmuls so no assembly DMA sits
            # on the critical path, and warm matmuls keeping the clock up ----
            def emit_tail(b, qc, q_lo, q_w, avs):
                attn_h = []
                for h in range(2):
                    bcp = ps_misc.tile([64, 512], F32, tag="misc", name="bc")[:, 0:q_w]
                    nc.tensor.matmul(
                        bcp, ones64[64:65, :], avs[h][64:65, :],
                        start=True, stop=True,
                    )
                    rbc = small.tile([64, 512], F32, tag="rbc", name="rbc")[:, 0:q_w]
                    nc.vector.reciprocal_approx_fast(rbc, bcp)
                    a = attnp.tile([64, 512], BF16, tag="attn1", name="attn_h")[:, 0:q_w]
                    nc.vector.tensor_mul(a, avs[h][0:64, :], rbc)
                    attn_h.append(a)
                    warm_fill(2, pool=ps_s)
                warm_fill(8, pool=ps_s)
                for tt in range(q_w // 128):
                    ysb = yout.tile([128, C], BF16, tag="ysb", name="ysb")
                    t0 = b * T + qc * 512 + q_lo + tt * 128
                    for cc in range(2):
                        yp = ps_misc.tile([128, 512], F32, tag="misc", name="yp")
                        for h in range(2):
                            w_h = (
                                wp_sb[0:64, cc * 512 : (cc + 1) * 512]
                                if h == 0
                                else wp_h1[:, cc * 512 : (cc + 1) * 512]
                            )
                            nc.tensor.matmul(
                                yp,
                                attn_h[h][:, tt * 128 : (tt + 1) * 128],
                                w_h,
                                start=(h == 0),
                                stop=(h == 1),
                            )
                        cs = slice(cc * 512, (cc + 1) * 512)
                        if cc == 0:
                            nc.vector.tensor_copy(out=ysb[:, cs], in_=yp)
                        else:
                            nc.scalar.copy(ysb[:, cs], yp)
                        # per-half y DMA so the final transfer starts earlier
                        nc.sync.dma_start(out=y[t0 : t0 + 128, cs], in_=ysb[:, cs])

            # ---- main pipeline ----
            # batch-interleaved chunk order: sizes grow monotonically
            # (4,4,8,8,12,12,16,16 k-tiles), so each chunk's deferred
            # normalize/projection work lands in an equal-or-larger
            # successor instead of bursting into a small one
            entries = [(b, qc, 0, 512) for qc in range(4) for b in range(B)]
            NB = 8  # qkv bundles, one per 512-token tile; bundle i feeds
            bundle = [(bb, qq) for (bb, qq, _, _) in entries[:NB]]  # entry i

            ths0 = qkv_thunks(*bundle[0])
            ths0[0]()
            ths0[1]()
            build_bv_bc()  # after the first qk chain: bv has landed by now
            ths0[2]()
            ths0[3]()
            dma_xt(*bundle[1])

            last = len(entries) - 1
            for ci, (b, qc, q_lo, q_w) in enumerate(entries):
                if ci + 2 < NB:  # x prefetch for bundle ci+2
                    bn, tjn = bundle[ci + 2]
                    pending.append((ci, lambda bn=bn, tjn=tjn: dma_xt(bn, tjn)))
                if ci + 1 < NB:  # qkv bundle ci+1, due before entry ci+1
                    bn, tjn = bundle[ci + 1]
                    for th in qkv_thunks(bn, tjn):
                        pending.append((ci + 1, th))
                is_tail = TAIL_INLINE and ci == last
                avs = emit_chunk(ci, b, qc, q_lo, q_w, tail=is_tail)
                if is_tail:
                    drain(ci + 10)
                    emit_tail(b, qc, q_lo, q_w, avs)
                else:
                    box = {}
                    pending.append((ci + 2, bc_norm_thunk(avs, q_w, box)))
                    # warm spacer: PE filler popped between the normalize
                    # chain and the first projection so its LDWEIGHTS does
                    # not stall on the DVE mul finishing
                    pending.append((ci + 2, lambda: warm_fill(1, pool=ps_s)))
                    for tt in range(q_w // 128):
                        ybox = {}
                        for cc in range(2):
                            pending.append(
                                (ci + 2, proj_thunk(b, qc, q_lo, tt, cc, box, ybox))
                            )
            drain(len(entries) + 2)
            assert not pending

    nc.compile()
    return nc


def _get_nc():
    if "nc" not in _CACHE:
        _CACHE["nc"] = _build()
    return _CACHE["nc"]


def _bf16(x: np.ndarray) -> np.ndarray:
    return np.ascontiguousarray(x).astype(ml_dtypes.bfloat16)


def _run(inputs, **spmd_kwargs):
    x = np.asarray(inputs["x"], dtype=np.float32)
    w_qkv = np.asarray(inputs["w_qkv"], dtype=np.float32)
    b_qkv = np.asarray(inputs["b_qkv"], dtype=np.float32)
    w_proj = np.asarray(inputs["w_proj"], dtype=np.float32)
    b_proj = np.asarray(inputs["b_proj"], dtype=np.float32)

    nc = _get_nc()

    xT = _bf16(x.reshape(N, C).T)
    in_maps = []
    for i in range(NCORES):
        f0 = i * FPC
        in_maps.append(
            {
                "xT": xT,
                "wq": _bf16(w_qkv[:, f0 : f0 + FPC]),
                "wk": _bf16(w_qkv[:, C + f0 : C + f0 + FPC]),
                "wv": _bf16(w_qkv[:, 2 * C + f0 : 2 * C + f0 + FPC]),
                "bq": np.ascontiguousarray(
                    b_qkv[f0 : f0 + FPC], dtype=np.float32
                ).reshape(FPC, 1),
                "bk": np.ascontiguousarray(
                    b_qkv[C + f0 : C + f0 + FPC], dtype=np.float32
                ).reshape(FPC, 1),
                "bv": _bf16(b_qkv[2 * C + f0 : 2 * C + f0 + FPC]).reshape(1, FPC),
                "wp": _bf16(w_proj[f0 : f0 + FPC, :]),
            }
        )

    res = run_bass_kernel_spmd(nc, in_maps, core_ids=list(range(NCORES)), **spmd_kwargs)
    acc = np.zeros((N, C), dtype=np.float64)
    for om in res.results:
        acc += np.asarray(om["y"]).astype(np.float64)
    out = (acc + b_proj.astype(np.float64)).astype(np.float32)
    return out.reshape(B, T, C), res


def kernel(**inputs) -> np.ndarray:
    out, _ = _run(inputs)
    return out


# revision 49
# speedup vs baseline: 1.1791x; 1.1791x over previous
"""Causal self-attention (B=2, T=2048, C=1024, H=16) on 8 TRN2 NeuronCores.

Sharding: tensor-parallel over heads. Each core owns 2 heads: it computes
q/k/v projections for its 128 feature columns, full causal attention for its
(batch, head) pairs, and a partial output projection against its 128 rows of
w_proj. The 8 partial [4096, 1024] outputs are summed on host and b_proj is
added once during that reduction.

v4: engine-specialized, bank-interleaved pipeline (from v3's trace):

  * ScalarE runs exp activations plus half the y evacuations only; causal
    masks + the attn assembly DMA go to the otherwise-idle GPSIMD engine;
  * every accumulation chain alternates between two PSUM banks (q/k
    projections pairwise-interleaved, V's four token-tile chains split
    across two tiles) so back-to-back matmuls pipeline their ~165ns
    fill/drain instead of serializing it;
  * the V bias is applied by a tensor_tensor add against a pre-broadcast
    bias tile during PSUM evacuation (no K=1 bias matmuls);
  * the final entry's normalize/projection runs inline with split-head
    (K=64 x2) projection matmuls -- no SBUF->SBUF assembly DMA on the tail
    critical path -- and warm dummy matmuls keep the PE p-state at full
    clock through the normalize latency;
  * first x tile DMA'd in four pieces so the first qkv chain starts ~1us in.

Softmax skips the max-subtraction: scores are ~N(0,1) (bounded ~+-6), far
inside fp32 exp range. Denominators fall out of the AV matmul via a ones
column appended to V per head; normalization happens on the tiny attn^T
tensor via a K=1 broadcast matmul + fast reciprocal.
"""

from collections import deque

import numpy as np
import ml_dtypes

import concourse.bass as bass
import concourse.mybir as mybir
import concourse.tile as tile
from concourse import bacc
from concourse.bass_utils import run_bass_kernel_spmd

F32 = mybir.dt.float32
BF16 = mybir.dt.bfloat16
EXP = mybir.ActivationFunctionType.Exp

B, T, C = 2, 2048, 1024
H, DH = 16, 64
NCORES = 8
FPC = (H // NCORES) * DH  # 128 q/k/v feature columns per core (2 heads)
N = B * T                 # 4096 tokens
NTT = N // 128            # 32 global 128-token tiles
NCT = C // 128            # 8 contraction tiles
SCALE = DH ** -0.5

# feature flags (HW bisect)
INTERLEAVE_QKV = True  # q/k and V chains alternate PSUM banks
BIAS_IN_EVAC = True    # V bias folded into evacuation add (no K=1 matmuls)
GPSIMD_OFFLOAD = True  # masks + norm-muls + assembly DMA on GPSIMD
TAIL_INLINE = True     # last entry: inline split-head proj + warm fill
LAG2 = True            # AV issued two steps after its S (else one step)

_CACHE = {}


def _build():
    nc = bacc.Bacc(
        "TRN2",
        target_bir_lowering=False,
        debug=False,
        enable_asserts=True,
        num_devices=NCORES,
    )
    xT = nc.dram_tensor("xT", [C, N], BF16, kind="ExternalInput").ap()
    wq = nc.dram_tensor("wq", [C, FPC], BF16, kind="ExternalInput").ap()
    wk = nc.dram_tensor("wk", [C, FPC], BF16, kind="ExternalInput").ap()
    wv = nc.dram_tensor("wv", [C, FPC], BF16, kind="ExternalInput").ap()
    bq = nc.dram_tensor("bq", [FPC, 1], F32, kind="ExternalInput").ap()
    bk = nc.dram_tensor("bk", [FPC, 1], F32, kind="ExternalInput").ap()
    bv = nc.dram_tensor("bv", [1, FPC], BF16, kind="ExternalInput").ap()
    wp = nc.dram_tensor("wp", [FPC, C], BF16, kind="ExternalInput").ap()
    y = nc.dram_tensor("y", [N, C], BF16, kind="ExternalOutput").ap()

    with tile.TileContext(nc) as tc:
        with (
            tc.tile_pool(name="const", bufs=1) as cst,
            tc.tile_pool(name="qkvt", bufs=1) as qkvt,
            tc.tile_pool(name="xin", bufs=3) as xin,
            tc.tile_pool(name="ptile", bufs=6) as ptile,
            tc.tile_pool(name="attn", bufs=12) as attnp,
            tc.tile_pool(name="yout", bufs=6) as yout,
            tc.tile_pool(name="small", bufs=3) as small,
            tc.tile_pool(name="ps_s", bufs=2, space="PSUM") as ps_s,
            tc.tile_pool(name="ps_av", bufs=2, space="PSUM") as ps_av,
            tc.tile_pool(name="ps_misc", bufs=2, space="PSUM") as ps_misc,
        ):
            # ---- input DMAs; first x tile split in quarters so the very
            # first qkv matmuls can start during the p-state ramp ----
            xT_view = xT.rearrange("(ct p) t -> p ct t", p=128)
            xt_tiles = {}

            def dma_xt(b, tj, split=False, eng=None):
                xt = xin.tile([128, NCT, 512], BF16, tag="xt", name="xt")
                t0 = b * T + tj * 512
                if split:
                    for i in range(4):
                        nc.sync.dma_start(
                            out=xt[:, 2 * i : 2 * i + 2, :],
                            in_=xT_view[:, 2 * i : 2 * i + 2, t0 : t0 + 512],
                        )
                else:
                    (eng or nc.sync).dma_start(
                        out=xt, in_=xT_view[:, :, t0 : t0 + 512]
                    )
                xt_tiles[(b, tj)] = xt

            # ---- weights / biases. DMA *issue* costs 0.6-3.5us of the
            # issuing engine's time, so spread issues across idle engine
            # queues: sync carries wq + the x stream (critical path), the
            # other engines carry everything else in parallel ----
            w_sb = {}
            w_sb["q"] = cst.tile([128, NCT, FPC], BF16, tag="wq", name="wq")
            nc.sync.dma_start(
                out=w_sb["q"], in_=wq.rearrange("(ct p) f -> p ct f", p=128)
            )
            dma_xt(0, 0, split=True)
            for name, wap, eng in (("k", wk, nc.scalar), ("v", wv, nc.scalar)):
                w_sb[name] = cst.tile(
                    [128, NCT, FPC], BF16, tag=f"w{name}", name=f"w{name}"
                )
                eng.dma_start(
                    out=w_sb[name], in_=wap.rearrange("(ct p) f -> p ct f", p=128)
                )
            b_sb = {}
            for name, bap in (("q", bq), ("k", bk)):
                b_sb[name] = cst.tile([FPC, 1], F32, tag=f"b{name}", name=f"b{name}")
                nc.scalar.dma_start(out=b_sb[name], in_=bap)
            bv_sb = cst.tile([1, FPC], BF16, tag="bv", name="bv")
            nc.gpsimd.dma_start(out=bv_sb, in_=bv)
            wp_sb = cst.tile([FPC, C], BF16, tag="wp", name="wp")
            nc.gpsimd.dma_start(out=wp_sb, in_=wp)
            # head-1 rows of w_proj staged at partitions 0-63 so the tail's
            # split-head projection has matching base partitions
            wp_h1 = cst.tile([64, C], BF16, tag="wph1", name="wph1")
            nc.gpsimd.dma_start(out=wp_h1, in_=wp[64:128, :])

            # ---- constants; warm-up deps (onesv/warmb) first so the PE's
            # dummy ramp matmuls can start as early as possible ----
            onesf = cst.tile([128, 128], F32, tag="onesf", name="onesf")
            nc.vector.memset(onesf, 1.0)
            onesv = cst.tile([1, 128], BF16, tag="onesv", name="onesv")
            nc.vector.tensor_copy(out=onesv, in_=onesf[0:1, :])
            warm = cst.tile([1, 512], F32, tag="warm", name="warm")
            nc.vector.memset(warm, 1.0)
            warmb = cst.tile([1, 512], BF16, tag="warmb", name="warmb")
            nc.vector.tensor_copy(out=warmb, in_=warm)

            def warm_fill(n, pool=None, width=512):
                for _ in range(n):
                    p = pool or ps_misc
                    wps = p.tile([128, width], F32, tag="misc" if p is ps_misc else "s",
                                 name="warmup")
                    nc.tensor.matmul(
                        wps, onesv, warmb[:, 0:width], start=True, stop=True
                    )

            warm_fill(10)
            # lower-triangle mask (keep iff q-col >= k-partition), duplicated
            # side by side so one strided multiply covers both heads
            mtf = cst.tile([128, 128], F32, tag="mtf", name="mtf")
            nc.vector.memset(mtf, 1.0)
            nc.gpsimd.affine_select(
                out=mtf,
                in_=mtf,
                compare_op=mybir.AluOpType.is_ge,
                fill=0.0,
                base=0,
                pattern=[[1, 128]],
                channel_multiplier=-1,
            )
            M_tri2 = cst.tile([128, 2, 128], BF16, tag="mtri", name="mtri")
            nc.vector.tensor_copy(out=M_tri2[:, 0, :], in_=mtf)
            nc.vector.tensor_copy(out=M_tri2[:, 1, :], in_=mtf)
            # ones row at partition 64 (stationary of the K=1 denominator
            # broadcast matmul; partition 64 = where AV's ones-column lands)
            ones64 = cst.tile([128, 64], BF16, tag="ones64", name="ones64")
            nc.vector.tensor_copy(out=ones64[64:65, :], in_=onesf[64:65, 0:64])
            bv_bc = cst.tile([128, 2, 2, 64], BF16, tag="bvbc", name="bvbc")

            def build_bv_bc():
                # V bias broadcast to all 128 token partitions, duplicated
                # for two token-tiles: [128, tt2, head2, 64] so one
                # tensor_tensor add covers a whole pv evacuation
                bvp = ps_misc.tile([128, 128], F32, tag="misc", name="bvp")
                nc.tensor.matmul(bvp, onesv, bv_sb, start=True, stop=True)
                nc.vector.tensor_copy(
                    out=bv_bc[:, 0, :, :],
                    in_=bvp.rearrange("p (two c) -> p two c", two=2),
                )
                nc.vector.tensor_copy(out=bv_bc[:, 1, :, :], in_=bv_bc[:, 0, :, :])

            # ---- persistent activations ----
            QT = qkvt.tile([FPC, N], BF16, tag="QT", name="QT")
            KT = qkvt.tile([FPC, N], BF16, tag="KT", name="KT")
            # V with a ones column per head: per 128-token tile block of 130
            # cols: [64 V_h0 | 1 | 64 V_h1 | 1]
            V = qkvt.tile([128, NTT * 130], BF16, tag="V", name="V")
            V_blk = V.rearrange("p (kt two c) -> p kt two c", two=2, c=65)
            nc.vector.tensor_copy(out=V_blk[:, :, 0, 64], in_=onesf[:, 0:NTT])
            nc.vector.tensor_copy(out=V_blk[:, :, 1, 64], in_=onesf[:, 0:NTT])

            # ---- deferred-work queue: (deadline_entry, thunk) ----
            pending = deque()

            def drain(ci):
                rest = [item for item in pending if item[0] > ci]
                due = [item for item in pending if item[0] <= ci]
                pending.clear()
                pending.extend(rest)
                for _, th in due:
                    th()

            def pop_some(steps_left):
                # pop faster when far behind early in an entry (small early
                # entries otherwise emit a bundle's k-projection evacuation
                # just before the S matmul that needs it); stay at 2 near
                # entry ends to avoid bursts
                if steps_left >= 4 and len(pending) > 2 * steps_left:
                    n = 3
                elif len(pending) > steps_left:
                    n = 2
                else:
                    n = 1 if pending else 0
                for _ in range(min(n, len(pending))):
                    pending.popleft()[1]()

            # ---- qkv projection bundle -> thunks ----
            # q and k accumulate pairwise-interleaved into two PSUM tiles so
            # consecutive matmuls hit different banks and pipeline; same for
            # V's four token-tile chains (tt0/tt1 in one tile, tt2/tt3 in the
            # other, ct-major emission alternating tiles).
            def qkv_thunks(b, tj):
                t0 = b * T + tj * 512
                qk_box = {}
                v_box = {}

                def qk_half(first):
                    def th():
                        xt = xt_tiles[(b, tj)]
                        if first:
                            qk_box["q"] = ps_misc.tile(
                                [128, 512], F32, tag="misc", name="accq"
                            )
                            qk_box["k"] = ps_misc.tile(
                                [128, 512], F32, tag="misc", name="acck"
                            )
                        cts = range(0, 4) if first else range(4, NCT)
                        for ct in cts:
                            for name in ("q", "k"):
                                nc.tensor.matmul(
                                    qk_box[name],
                                    w_sb[name][:, ct, :],
                                    xt[:, ct, :],
                                    start=(ct == 0),
                                    stop=(ct == NCT - 1),
                                    skip_group_check=True,
                                )
                        if not first:
                            nc.vector.tensor_scalar_add(
                                QT[:, t0 : t0 + 512], qk_box["q"], b_sb["q"]
                            )
                            nc.vector.tensor_scalar_add(
                                KT[:, t0 : t0 + 512], qk_box["k"], b_sb["k"]
                            )

                    return th

                def v_half(first):
                    # a PSUM bank supports only ONE open accumulation group:
                    # interleave strictly ACROSS the two tiles (banks), with
                    # each pass running complete chains (tt0/tt2 first, then
                    # tt1/tt3 in the tiles' second halves)
                    def th():
                        xt = xt_tiles[(b, tj)]
                        if first:
                            v_box["a"] = ps_misc.tile(
                                [128, 256], F32, tag="misc", name="pva"
                            )
                            v_box["b"] = ps_misc.tile(
                                [128, 256], F32, tag="misc", name="pvb"
                            )
                        pairs = (
                            ((0, v_box["a"], slice(0, 128)),
                             (2, v_box["b"], slice(0, 128)))
                            if first
                            else ((1, v_box["a"], slice(128, 256)),
                                  (3, v_box["b"], slice(128, 256)))
                        )
                        for ct in range(NCT):
                            for tt, tile_, sl in pairs:
                                nc.tensor.matmul(
                                    tile_[:, sl],
                                    xt[:, ct, tt * 128 : (tt + 1) * 128],
                                    w_sb["v"][:, ct, :],
                                    start=(ct == 0),
                                    stop=(ct == NCT - 1),
                                    skip_group_check=True,
                                )
                        if not first:
                            gt = b * 16 + tj * 4
                            for key, g0 in (("a", gt), ("b", gt + 2)):
                                pv = v_box[key].rearrange(
                                    "p (tt two c) -> p tt two c", tt=2, two=2, c=64
                                )
                                nc.vector.tensor_tensor(
                                    out=V_blk[:, g0 : g0 + 2, :, 0:64],
                                    in0=pv,
                                    in1=bv_bc,
                                    op=mybir.AluOpType.add,
                                )

                    return th

                return [qk_half(True), qk_half(False), v_half(True), v_half(False)]

            # ---- softmax-normalize + output projection (deferred) ----
            def bc_norm_thunk(avs, q_w, box):
                def th():
                    attn_t = attnp.tile([128, 512], BF16, tag="attn", name="attn")[:, 0:q_w]
                    attn1 = attnp.tile([64, 512], BF16, tag="attn1", name="attn1")[:, 0:q_w]
                    for h in range(2):
                        bcp = ps_misc.tile([64, 512], F32, tag="misc", name="bc")[:, 0:q_w]
                        nc.tensor.matmul(
                            bcp, ones64[64:65, :], avs[h][64:65, :],
                            start=True, stop=True,
                        )
                        rbc = small.tile([64, 512], F32, tag="rbc", name="rbc")[:, 0:q_w]
                        nc.vector.reciprocal_approx_fast(rbc, bcp)
                        tgt = attn_t[0:64, :] if h == 0 else attn1
                        nc.vector.tensor_mul(tgt, avs[h][0:64, :], rbc)
                    # head-1 half to partitions 64..127 (SBUF->SBUF DMA is the
                    # only cheap cross-partition path); issued from GPSIMD so
                    # ScalarE stays a pure exp stream
                    dma_eng = nc.gpsimd if GPSIMD_OFFLOAD else nc.scalar
                    dma_eng.dma_start(out=attn_t[64:128, :], in_=attn1)
                    box["attn"] = attn_t

                return th

            def proj_thunk(b, qc, q_lo, tt, cc, box, ybox):
                def th():
                    attn_t = box["attn"]
                    yp = ps_misc.tile([128, 512], F32, tag="misc", name="yp")
                    nc.tensor.matmul(
                        yp,
                        attn_t[:, tt * 128 : (tt + 1) * 128],
                        wp_sb[:, cc * 512 : (cc + 1) * 512],
                        start=True,
                        stop=True,
                    )
                    if cc == 0:
                        ybox["ysb"] = yout.tile([128, C], BF16, tag="ysb", name="ysb")
                    ysb = ybox["ysb"]
                    # evacuation alternates DVE / ScalarE so neither becomes
                    # the bottleneck
                    if cc == 0:
                        nc.vector.tensor_copy(
                            out=ysb[:, cc * 512 : (cc + 1) * 512], in_=yp
                        )
                    else:
                        nc.scalar.copy(ysb[:, cc * 512 : (cc + 1) * 512], yp)
                        t0 = b * T + qc * 512 + q_lo + tt * 128
                        nc.sync.dma_start(out=y[t0 : t0 + 128, :], in_=ysb)

                return th

            # ---- attention chunk (columns [q_lo, q_lo+q_w) of q-chunk qc):
            # S -> exp/mask -> AV (lag-2), deferred thunks sprinkled ----
            def emit_chunk(ci, b, qc, q_lo, q_w, tail=False):
                drain(ci)
                q0 = b * T + qc * 512 + q_lo
                hi = qc * 512 + q_lo + q_w  # global end col within the batch
                nkt = hi // 128
                av = [
                    ps_av.tile([65, 512], F32, tag="av", name="av")[:, 0:q_w]
                    for _ in range(2)
                ]

                def emit_av(pt, lo, kt):
                    gkt = b * 16 + kt
                    for h in range(2):
                        nc.tensor.matmul(
                            av[h][:, lo:q_w] if lo else av[h],
                            V[:, 130 * gkt + 65 * h : 130 * gkt + 65 * h + 65],
                            pt[:, h * q_w + lo : (h + 1) * q_w],
                            start=(kt == 0),
                            stop=(kt == nkt - 1),
                            skip_group_check=True,
                        )

                backlog = deque()
                for kt in range(nkt):
                    # valid cols of this k-tile within [q_lo, q_lo+q_w)
                    lo = min(max(kt * 128 - (qc * 512 + q_lo), 0), q_w)
                    diag = kt * 128 >= qc * 512 + q_lo  # triangle block here
                    k0 = b * T + kt * 128
                    # both heads in one S tile: the two halves land in
                    # different PSUM banks (512 f32 cols each) so the two
                    # matmuls pipeline, and one exp instruction (262ns fixed
                    # cost) covers both heads
                    s = ps_s.tile([128, 1024], F32, tag="s", name="s")[:, 0 : 2 * q_w]
                    for h in range(2):
                        nc.tensor.matmul(
                            s[:, h * q_w + lo : (h + 1) * q_w],
                            KT[64 * h : 64 * h + 64, k0 : k0 + 128],
                            QT[64 * h : 64 * h + 64, q0 + lo : q0 + q_w],
                            start=True,
                            stop=True,
                        )
                    pt = ptile.tile([128, 1024], BF16, tag="pt", name="pt")[:, 0 : 2 * q_w]
                    if lo == 0:
                        nc.scalar.activation(out=pt, in_=s, func=EXP, scale=SCALE)
                    else:
                        sv = s.rearrange("p (two c) -> p two c", two=2)
                        pv_ = pt.rearrange("p (two c) -> p two c", two=2)
                        nc.scalar.activation(
                            out=pv_[:, :, lo:q_w],
                            in_=sv[:, :, lo:q_w],
                            func=EXP,
                            scale=SCALE,
                        )
                    if diag:  # triangle crossing block, both heads (DVE:
                        # GPSIMD computes this wrong and is 2x slower)
                        ptv = pt.rearrange("p (two c) -> p two c", two=2)
                        nc.vector.tensor_mul(
                            ptv[:, :, lo : lo + 128],
                            ptv[:, :, lo : lo + 128],
                            M_tri2,
                        )
                    pop_some(nkt - 1 - kt)
                    backlog.append((pt, lo, kt))
                    if len(backlog) > (3 if LAG2 else 1):
                        emit_av(*backlog.popleft())
                while backlog:
                    emit_av(*backlog.popleft())
                # evacuate AV PSUM (numerators + denominators) to SBUF bf16;
                # the tail uses ScalarE (idle by then) so the DVE recip/mul
                # chain starts sooner
                avs = []
                for h in range(2):
                    a = attnp.tile([65, 512], BF16, tag="avs", name="avs")[:, 0:q_w]
                    if tail:
                        nc.scalar.copy(a, av[h])
                    else:
                        nc.vector.tensor_copy(a, av[h])
                    avs.append(a)
                return avs

            # ---- tail: the last entry's normalize + projection, inline,
            # with split-head (K=64 x2) proj matmuls so no assembly DMA sits
            # on the critical path, and warm matmuls keeping the clock up ----
            def emit_tail(b, qc, q_lo, q_w, avs):
                attn_h = []
                for h in range(2):
                    bcp = ps_misc.tile([64, 512], F32, tag="misc", name="bc")[:, 0:q_w]
                    nc.tensor.matmul(
                        bcp, ones64[64:65, :], avs[h][64:65, :],
                        start=True, stop=True,
                    )
                    rbc = small.tile([64, 512], F32, tag="rbc", name="rbc")[:, 0:q_w]
                    nc.vector.reciprocal_approx_fast(rbc, bcp)
                    a = attnp.tile([64, 512], BF16, tag="attn1", name="attn_h")[:, 0:q_w]
                    nc.vector.tensor_mul(a, avs[h][0:64, :], rbc)
                    attn_h.append(a)
                    warm_fill(2, pool=ps_s)
                warm_fill(8, pool=ps_s)
                for tt in range(q_w // 128):
                    ysb = yout.tile([128, C], BF16, tag="ysb", name="ysb")
                    t0 = b * T + qc * 512 + q_lo + tt * 128
                    for cc in range(2):
                        yp = ps_misc.tile([128, 512], F32, tag="misc", name="yp")
                        for h in range(2):
                            w_h = (
                                wp_sb[0:64, cc * 512 : (cc + 1) * 512]
                                if h == 0
                                else wp_h1[:, cc * 512 : (cc + 1) * 512]
                            )
                            nc.tensor.matmul(
                                yp,
                                attn_h[h][:, tt * 128 : (tt + 1) * 128],
                                w_h,
                                start=(h == 0),
                                stop=(h == 1),
                            )
                        cs = slice(cc * 512, (cc + 1) * 512)
                        if cc == 0:
                            nc.vector.tensor_copy(out=ysb[:, cs], in_=yp)
                        else:
                            nc.scalar.copy(ysb[:, cs], yp)
                        # per-half y DMA so the final transfer starts earlier
                        nc.sync.dma_start(out=y[t0 : t0 + 128, cs], in_=ysb[:, cs])

            # ---- main pipeline ----
            # batch-interleaved chunk order: sizes grow monotonically
            # (4,4,8,8,12,12,16,16 k-tiles), so each chunk's deferred
            # normalize/projection work lands in an equal-or-larger
            # successor instead of bursting into a small one
            entries = [(b, qc, 0, 512) for qc in range(4) for b in range(B)]
            NB = 8  # qkv bundles, one per 512-token tile; bundle i feeds
            bundle = [(bb, qq) for (bb, qq, _, _) in entries[:NB]]  # entry i

            ths0 = qkv_thunks(*bundle[0])
            ths0[0]()
            ths0[1]()
            build_bv_bc()  # after the first qk chain: bv has landed by now
            ths0[2]()
            ths0[3]()
            # issue from the gpsimd queue: sync is still busy issuing the
            # first tile's pieces, so this lands ~2us earlier
            dma_xt(*bundle[1], eng=nc.gpsimd)

            last = len(entries) - 1
            for ci, (b, qc, q_lo, q_w) in enumerate(entries):
                if ci + 2 < NB:  # x prefetch for bundle ci+2
                    bn, tjn = bundle[ci + 2]
                    pending.append((ci, lambda bn=bn, tjn=tjn: dma_xt(bn, tjn)))
                if ci + 1 < NB:  # qkv bundle ci+1, due before entry ci+1
                    bn, tjn = bundle[ci + 1]
                    for th in qkv_thunks(bn, tjn):
                        pending.append((ci + 1, th))
                is_tail = TAIL_INLINE and ci == last
                avs = emit_chunk(ci, b, qc, q_lo, q_w, tail=is_tail)
                if is_tail:
                    drain(ci + 10)
                    emit_tail(b, qc, q_lo, q_w, avs)
                else:
                    box = {}
                    pending.append((ci + 2, bc_norm_thunk(avs, q_w, box)))
                    for tt in range(q_w // 128):
                        ybox = {}
                        for cc in range(2):
                            pending.append(
                                (ci + 2, proj_thunk(b, qc, q_lo, tt, cc, box, ybox))
                            )
            drain(len(entries) + 2)
            assert not pending

    nc.compile()
    return nc


def _get_nc():
    if "nc" not in _CACHE:
        _CACHE["nc"] = _build()
    return _CACHE["nc"]


def _bf16(x: np.ndarray) -> np.ndarray:
    return np.ascontiguousarray(x).astype(ml_dtypes.bfloat16)


def _run(inputs, **spmd_kwargs):
    x = np.asarray(inputs["x"], dtype=np.float32)
    w_qkv = np.asarray(inputs["w_qkv"], dtype=np.float32)
    b_qkv = np.asarray(inputs["b_qkv"], dtype=np.float32)
    w_proj = np.asarray(inputs["w_proj"], dtype=np.float32)
    b_proj = np.asarray(inputs["b_proj"], dtype=np.float32)

    nc = _get_nc()

    xT = _bf16(x.reshape(N, C).T)
    in_maps = []
    for i in range(NCORES):
        f0 = i * FPC
        in_maps.append(
            {
                "xT": xT,
                "wq": _bf16(w_qkv[:, f0 : f0 + FPC]),
                "wk": _bf16(w_qkv[:, C + f0 : C + f0 + FPC]),
                "wv": _bf16(w_qkv[:, 2 * C + f0 : 2 * C + f0 + FPC]),
                "bq": np.ascontiguousarray(
                    b_qkv[f0 : f0 + FPC], dtype=np.float32
                ).reshape(FPC, 1),
                "bk": np.ascontiguousarray(
                    b_qkv[C + f0 : C + f0 + FPC], dtype=np.float32
                ).reshape(FPC, 1),
                "bv": _bf16(b_qkv[2 * C + f0 : 2 * C + f0 + FPC]).reshape(1, FPC),
                "wp": _bf16(w_proj[f0 : f0 + FPC, :]),
            }
        )

    res = run_bass_kernel_spmd(nc, in_maps, core_ids=list(range(NCORES)), **spmd_kwargs)
    acc = np.zeros((N, C), dtype=np.float64)
    for om in res.results:
        acc += np.asarray(om["y"]).astype(np.float64)
    out = (acc + b_proj.astype(np.float64)).astype(np.float32)
    return out.reshape(B, T, C), res


def kernel(**inputs) -> np.ndarray:
    out, _ = _run(inputs)
    return out
